# revision 3
# baseline (speedup 1.0000x reference)
"""AnchorAttention distributed Bass kernel for 8 TRN2 NeuronCores.

Sharding: core c handles batch b = c//2, row-half h = c%2 of that batch:
  - 128 anchor rows (of the batch's 256) + 1920 query rows = 2048 rows/core
  - anchor inputs x[b, :256] replicated within the pair (K/V need all 256)
No collectives: output shards disjoint; host reassembles.

vs the previous row-cyclic sharding this computes K/V for ONE batch per
core instead of four (saves ~98K PE cycles/core), and processes rows in
four 512-row chunks so projections, attention, and the output projection
of adjacent chunks pipeline across engines.

All device compute uses feature-on-partition ("transposed") layouts; the
host pre-transposes inputs.
"""
import sys

for _p in ("/opt/trn_rl_repo", "/root/.axon_site/_ro/trn_rl_repo"):
    if _p not in sys.path:
        sys.path.insert(0, _p)

import os
import numpy as np
import ml_dtypes

import concourse.bass as bass
import concourse.mybir as mybir
import concourse.tile as tile
from concourse import bacc
from concourse.bass_utils import run_bass_kernel_spmd

B, N, D = 4, 4096, 1024
H, HD = 16, 64
KA = 256                   # num anchor tokens per batch
NCORES = 8
AQ = KA // 2               # 128 anchor rows per core
QW = (N - KA) // 2         # 1920 query rows per core
R = AQ + QW                # 2048 output rows per core
CH = 512                   # chunk rows
NCH = R // CH              # 4 chunks
SCALE = 1.0 / float(np.sqrt(HD))

F32 = mybir.dt.float32
BF16 = mybir.dt.bfloat16
EXP = mybir.ActivationFunctionType.Exp

BF = ml_dtypes.bfloat16


def build_graph(repeat=1, cfg=None):
    nc = bacc.Bacc("TRN2", target_bir_lowering=False, debug=False,
                   num_devices=NCORES)

    # ---- external I/O (per-core shards) ----
    xaT_e = nc.dram_tensor("xaT", [D, KA], BF16, kind="ExternalInput")
    xamT_e = nc.dram_tensor("xamT", [D, AQ], BF16, kind="ExternalInput")
    xqT_e = nc.dram_tensor("xqT", [D, QW], BF16, kind="ExternalInput")
    wk_e = nc.dram_tensor("wk", [D, D], BF16, kind="ExternalInput")
    wv_e = nc.dram_tensor("wv", [D, D], BF16, kind="ExternalInput")
    wq_e = nc.dram_tensor("wq", [D, D], BF16, kind="ExternalInput")
    wqt_e = nc.dram_tensor("wqt", [D, D], BF16, kind="ExternalInput")
    wo_e = nc.dram_tensor("wo", [D, D], BF16, kind="ExternalInput")
    b3_e = nc.dram_tensor("b3_t", [128, 24], F32, kind="ExternalInput")
    b2_e = nc.dram_tensor("b2_r", [1, 2 * D], BF16, kind="ExternalInput")
    out_e = nc.dram_tensor("out", [R, D], BF16, kind="ExternalOutput")

    def wload(pool, ext, name):
        t = pool.tile([128, 8, D], BF16, name=name)
        nc.sync.dma_start(t[:], ext.rearrange("(o p) e -> p o e", p=128))
        return t

    with tile.TileContext(nc) as tc:
        with tc.tile_pool(name="perm", bufs=1) as perm, \
             tc.tile_pool(name="xq_stream", bufs=2) as pxq, \
             tc.tile_pool(name="q_pool", bufs=2) as pq, \
             tc.tile_pool(name="ctx_pool", bufs=13) as pctx, \
             tc.tile_pool(name="pool_p", bufs=5) as pool_p, \
             tc.tile_pool(name="pool_rec", bufs=5) as pool_rec, \
             tc.tile_pool(name="pool_rr", bufs=5) as pool_rr, \
             tc.tile_pool(name="pool_ot", bufs=2) as pool_ot, \
             tc.tile_pool(name="psum_proj", bufs=2, space="PSUM") as pp, \
             tc.tile_pool(name="ps_scores", bufs=2, space="PSUM") as psS, \
             tc.tile_pool(name="ps_ctx", bufs=3, space="PSUM") as psC, \
             tc.tile_pool(name="ps_out", bufs=1, space="PSUM") as psO:

          for _rep in range(repeat):

            # --- DMA priority order: small/critical first ---
            b3_sb = perm.tile([128, 24], F32)
            nc.sync.dma_start(b3_sb[:], b3_e[:])
            b2_sb = perm.tile([1, 2 * D], BF16)
            nc.sync.dma_start(b2_sb[:], b2_e[:])
            bq_sb, bk_sb, bqt_sb = b3_sb[:, 0:8], b3_sb[:, 8:16], b3_sb[:, 16:24]
            b2_bc = perm.tile([128, 2 * D], BF16)
            nc.gpsimd.partition_broadcast(b2_bc[:], b2_sb[:])
            bv_bc, bo_bc = b2_bc[:, 0:D], b2_bc[:, D:2 * D]

            xa_sb = perm.tile([128, 8, KA], BF16, name="xa_all")
            nc.sync.dma_start(xa_sb[:], xaT_e.rearrange("(o p) f -> p o f", p=128))
            xam_sb = perm.tile([128, 8, AQ], BF16, name="xam")
            nc.sync.dma_start(xam_sb[:], xamT_e.rearrange("(o p) f -> p o f", p=128))
            wk_sb = wload(perm, wk_e, "wk_sb")
            wv_sb = wload(perm, wv_e, "wv_sb")
            wq_sb = wload(perm, wq_e, "wq_sb")
            wqt_sb = wload(perm, wqt_e, "wqt_sb")

            xq_chunks = {}

            def load_xq(ch):
                if ch >= NCH:
                    return
                t = pxq.tile([128, 8, CH], BF16, tag="xq", name=f"xq{ch}")
                if ch == 0:
                    nc.sync.dma_start(
                        t[:, :, 0:QW - 3 * CH],
                        xqT_e.rearrange("(o p) f -> p o f", p=128)
                        [:, :, 0:QW - 3 * CH])
                else:
                    lo = QW - 3 * CH + (ch - 1) * CH
                    nc.sync.dma_start(
                        t[:],
                        xqT_e.rearrange("(o p) f -> p o f", p=128)
                        [:, :, lo:lo + CH])
                xq_chunks[ch] = t

            load_xq(0)
            wo_sb = wload(perm, wo_e, "wo_sb")

            # --- K projection: kT [128e, 8et, 256 keys] ---
            kT = perm.tile([128, 8, KA], BF16, name="kT")
            for et in range(8):
                psf = pp.tile([128, 512], F32, tag="proj", name="psk")
                ps = psf[:, :KA]
                for dt in range(8):
                    nc.tensor.matmul(
                        ps, wk_sb[:, dt, et * 128:(et + 1) * 128],
                        xa_sb[:, dt, :], start=(dt == 0), stop=(dt == 7))
                nc.scalar.add(kT[:, et, :], ps, bk_sb[:, et:et + 1])

            # --- V projection: v [128 keys(2at), 16h, HD+1], ones col last
            #     so the ctx matmul also produces the softmax denominator
            #     at psum partition HD ---
            v_b = perm.tile([128, 2, H, HD + 1], BF16, name="v_b")
            nc.vector.memset(v_b[:, :, :, HD:HD + 1], 1.0)
            dof = 0
            for at in range(2):
                for en in range(2):
                    ps = pp.tile([128, 512], F32, tag="proj", name="psv")
                    for dt in range(8):
                        nc.tensor.matmul(
                            ps, xa_sb[:, dt, at * 128:(at + 1) * 128],
                            wv_sb[:, dt, en * 512:(en + 1) * 512],
                            start=(dt == 0), stop=(dt == 7))
                    nc.vector.tensor_add(
                        v_b[:, at, en * 8:(en + 1) * 8, dof:dof + HD],
                        ps.rearrange("p (h x) -> p h x", x=HD),
                        bv_bc[:, en * 512:(en + 1) * 512].rearrange(
                            "p (h x) -> p h x", x=HD))

            # --- per-chunk pipeline (software-pipelined emission) ---
            def emit_qa(qT):
                # anchor rows 0:128 via Wq from this half's anchors
                for et in range(8):
                    psf = pp.tile([128, 512], F32, tag="proj", name="psqa")
                    ps = psf[:, :AQ]
                    for dt in range(8):
                        nc.tensor.matmul(
                            ps, wq_sb[:, dt, et * 128:(et + 1) * 128],
                            xam_sb[:, dt, :],
                            start=(dt == 0), stop=(dt == 7))
                    nc.vector.tensor_scalar_add(
                        qT[:, et, 0:AQ], ps, bq_sb[:, et:et + 1])

            def emit_qt_groups(ch, qT):
                """Generator: one Wqt-projection e-tile group per yield."""
                load_xq(ch + 1)
                qoff = AQ if ch == 0 else 0
                qn = CH - qoff
                for et in range(8):
                    psf = pp.tile([128, 512], F32, tag="proj", name="psq")
                    ps = psf[:, :qn]
                    for dt in range(8):
                        nc.tensor.matmul(
                            ps, wqt_sb[:, dt, et * 128:(et + 1) * 128],
                            xq_chunks[ch][:, dt, 0:qn],
                            start=(dt == 0), stop=(dt == 7))
                    nc.scalar.add(qT[:, et, qoff:qoff + qn], ps,
                                  bqt_sb[:, et:et + 1])
                    yield

            pair_scores = os.environ.get("KV_PAIR", "1") == "1"

            def emit_tail(ps_c, dst, use_act_copy):
                # sum sits at psum partition 64 (base-64 reads are legal
                # only for 1-partition copies; recip/broadcast need
                # base-0). Copy to sbuf@0 (alternating Act/DVE for
                # balance), recip on DVE, broadcast on gpsimd; the
                # normalize-mul is the psum evacuation.
                rr = pool_rr.tile([1, 2, 512], F32, tag="rr")
                rec = pool_rec.tile([64, 512], F32, tag="rec")
                if use_act_copy:
                    nc.scalar.copy(rr[:, 0, :], ps_c[HD:HD + 1, :])
                else:
                    nc.vector.tensor_copy(rr[:, 0, :], ps_c[HD:HD + 1, :])
                nc.vector.reciprocal_approx_fast(rr[:, 1, :], rr[:, 0, :])
                nc.gpsimd.partition_broadcast(rec[:], rr[:, 1, :])
                nc.vector.tensor_mul(dst, ps_c[0:HD, :], rec[:])

            def emit_attention(ch, qT, ctx_ts):
                """Generator: per yield, one (et, par) instance (unpaired)
                or one et = two head instances (paired emission: the four
                64-partition score matmuls go out adjacently so the two
                row-groups overlap in the PE array on hardware)."""
                for et in range(8):
                    if pair_scores:
                        p_t = {}
                        for at in range(2):
                            for par in range(2):
                                po = par * 64
                                ps_s = psS.tile([128, 512], F32, tag="s")
                                nc.tensor.matmul(
                                    ps_s,
                                    kT[po:po + 64, et,
                                       at * 128:(at + 1) * 128],
                                    qT[po:po + 64, et, :],
                                    start=True, stop=True,
                                    tile_position=(po, 0))
                                pt = pool_p.tile([128, 512], BF16, tag="p")
                                nc.scalar.activation(pt[:], ps_s, EXP,
                                                     scale=SCALE)
                                p_t[at, par] = pt
                        for par in range(2):
                            po = par * 64
                            h = 2 * et + par
                            ps_c = psC.tile([128, 512], F32, tag="c")
                            for at in range(2):
                                nc.tensor.matmul(
                                    ps_c[0:HD + 1, :], v_b[:, at, h, :],
                                    p_t[at, par][:],
                                    start=(at == 0), stop=(at == 1),
                                    tile_position=(0, 0))
                            emit_tail(ps_c, ctx_ts[et][po:po + 64, :],
                                      use_act_copy=(par == 0))
                        yield
                    else:
                        for par in range(2):
                            po = par * 64
                            h = 2 * et + par
                            ps_c = psC.tile([128, 512], F32, tag="c")
                            p_t = {}
                            for at in range(2):
                                ps_s = psS.tile([128, 512], F32, tag="s")
                                nc.tensor.matmul(
                                    ps_s,
                                    kT[po:po + 64, et,
                                       at * 128:(at + 1) * 128],
                                    qT[po:po + 64, et, :],
                                    start=True, stop=True,
                                    tile_position=(po, 0))
                                pt = pool_p.tile([128, 512], BF16, tag="p")
                                nc.scalar.activation(pt[:], ps_s, EXP,
                                                     scale=SCALE)
                                p_t[at] = pt
                            for at in range(2):
                                nc.tensor.matmul(
                                    ps_c[0:HD + 1, :], v_b[:, at, h, :],
                                    p_t[at][:],
                                    start=(at == 0), stop=(at == 1),
                                    tile_position=(0, 0))
                            emit_tail(ps_c, ctx_ts[et][po:po + 64, :],
                                      use_act_copy=(par == 0))
                            yield

            def emit_out_groups(ch, ctx_ts):
                """Generator: one (rt, eo) output-projection group per yield."""
                for rt in range(4):
                    ot = pool_ot.tile([128, D], BF16, tag="ot")
                    for eo in range(2):
                        ps_o = psO.tile([128, 512], F32, tag="o")
                        for ct in range(8):
                            nc.tensor.matmul(
                                ps_o, ctx_ts[ct][:, rt * 128:(rt + 1) * 128],
                                wo_sb[:, ct, eo * 512:(eo + 1) * 512],
                                start=(ct == 0), stop=(ct == 7))
                        nc.vector.tensor_add(
                            ot[:, eo * 512:(eo + 1) * 512], ps_o,
                            bo_bc[:, eo * 512:(eo + 1) * 512])
                        yield
                    nc.sync.dma_start(
                        out_e[ch * CH + rt * 128:ch * CH + (rt + 1) * 128, :],
                        ot[:])

            def drain(*gens):
                gens = list(gens)
                while gens:
                    for g in list(gens):
                        try:
                            next(g)
                        except StopIteration:
                            gens.remove(g)

            # 3-stage software pipeline: attention(ch) runs interleaved with
            # the next chunk's q-projection and the previous chunk's output
            # projection, so PE always has independent matmuls to issue
            # while softmax tails and psum evacuations drain.
            qTs, ctxs = {}, {}
            qTs[0] = pq.tile([128, 8, CH], BF16, tag="qT", name="qT0")
            emit_qa(qTs[0])
            drain(emit_qt_groups(0, qTs[0]))
            for ch in range(NCH + 1):
                gens = []
                if ch < NCH:
                    ctxs[ch] = [pctx.tile([128, CH], BF16, tag="ctxT",
                                          name=f"ctxT{ch}_{i}")
                                for i in range(8)]
                    gens.append(emit_attention(ch, qTs[ch], ctxs[ch]))
                if ch + 1 < NCH:
                    qTs[ch + 1] = pq.tile([128, 8, CH], BF16, tag="qT",
                                          name=f"qT{ch + 1}")
                    gens.append(emit_qt_groups(ch + 1, qTs[ch + 1]))
                if ch >= 1:
                    gens.append(emit_out_groups(ch - 1, ctxs[ch - 1]))
                drain(*gens)

    nc.compile()
    return nc


def host_prep(x, Wq, bq, Wk, bk, Wv, bv, Wqt, bqt, Wo, bo):
    """Build per-core in_maps from full inputs."""
    x = np.asarray(x, dtype=np.float32)
    bf = lambda a: np.ascontiguousarray(np.asarray(a, np.float32)).astype(BF)
    bias_t = lambda v: np.asarray(v, np.float32).reshape(8, 128).T  # [128, 8]
    b3 = np.ascontiguousarray(
        np.concatenate([bias_t(bq), bias_t(bk), bias_t(bqt)], axis=1))
    b2 = np.concatenate([np.asarray(bv, np.float32),
                         np.asarray(bo, np.float32)]).reshape(1, 2 * D)
    common = {
        "wk": bf(Wk), "wv": bf(Wv), "wqt": bf(Wqt),
        "wq": bf(Wq), "wo": bf(Wo),
        "b3_t": b3, "b2_r": b2.astype(BF),
    }
    in_maps = []
    for c in range(NCORES):
        b, h = c // 2, c % 2
        xa = x[b, :KA, :]                                  # [KA, D]
        xaT = np.ascontiguousarray(xa.T).astype(BF)        # [D, KA]
        xq = x[b, KA + h * QW:KA + (h + 1) * QW, :]        # [QW, D]
        xqT = np.ascontiguousarray(xq.T).astype(BF)        # [D, QW]
        xamT = np.ascontiguousarray(xaT[:, h * AQ:(h + 1) * AQ])
        in_maps.append(dict(common, xaT=xaT, xqT=xqT, xamT=xamT))
    return in_maps


def assemble(results):
    """[core][r, e] shards -> full [B, N, D]."""
    out = np.empty((B, N, D), dtype=np.float32)
    for c in range(NCORES):
        b, h = c // 2, c % 2
        o = np.asarray(results[c]["out"], dtype=np.float32)
        out[b, h * AQ:(h + 1) * AQ] = o[:AQ]
        out[b, KA + h * QW:KA + (h + 1) * QW] = o[AQ:]
    return out


def kernel(x, Wq, bq, Wk, bk, Wv, bv, Wqt, bqt, Wo, bo, num_anchor_tokens):
    assert int(num_anchor_tokens) == KA, f"expected {KA} anchors"
    in_maps = host_prep(x, Wq, bq, Wk, bk, Wv, bv, Wqt, bqt, Wo, bo)
    nc = build_graph()
    res = run_bass_kernel_spmd(nc, in_maps, core_ids=list(range(NCORES)))
    return assemble(res.results)
